# revision 44
# baseline (speedup 1.0000x reference)
"""Trainium2 Bass kernel for nn_AttentionTSSA (B=8, N=8192, C=512, H=8).

Sharding: data-parallel over batch B across the 8 NeuronCores (1 batch each,
no collectives).  Per core (all 16-bit tensors are fp16: same engine speeds
as bf16 but 8x the mantissa — total rel err ~7e-4):

  phase 1:  wT[c, n] = Wqkv @ x^T (fp16 GEMM, 512-token chunks, one batched
            x DMA per chunk).  Each PSUM bank is evacuated twice: DVE copies
            to fp16 wt (kept resident), ScalarE squares to fp16 w2 with
            accum_out giving per-channel norm^2 partials for free.
  finalize: inv[c] = 1/max(norm^2, eps); amat[ci] [128,16] fp16 packs
            inv*temp (cols 0:8) and ones (cols 8:16).
  phase 2a: per 128-token tile, matmul(w2 tile, amat) -> psB[tok, 16] =
            (sum_ws*temp | r).  With temp==ones the mask multiply alone
            reproduces the reference masked softmax (uniform 1/8), so the
            softmax is: one masked TT, one Exp, reduce, recip, one TT.
            sum(Pi) / sum(Pi*r) accumulate into a single psS bank via
            erec-stationary matmuls (the bank is memset once and accumulated
            with start=False only: an interleaved start=True wipes other
            open groups' partials in the same bank — HW-verified).  The PE
            consumers of the softmax chain lag LAG chunks so PE never stalls.
  scalars:  dots/attn from psS; -attn[h] expanded per channel partition via
            tiny ind8 matmuls (wex[ci]).
  phase 3:  per 2-chunk pair, Pi transposes to head-major pitc; per chunk,
            psE[c, n] = Pi[h(c), n] by a PE indicator matmul (a partition-
            broadcast DMA measures ~8us/transfer on this HW — avoid), then
            wts = wt * (-attn[h(c)]) * Pi in one scalar_tensor_tensor; the
            output GEMM outT = WoutT^T @ wts with bias fused into the PSUM
            evacuation; one batched fp16 output DMA per chunk (host upcasts).

Timing note: unrolled multi-rep NEFFs are instruction-fetch bound on this
part (~4x slowdown at 33 reps); test.py measures with an on-device tc.For_i
hardware loop instead.

Host side transposes x per batch and un-transposes/upcasts the outputs.
"""

import numpy as np

B, N, C, H = 8, 8192, 512, 8
D = C // H          # 64
CT = C // 128       # 4 channel tiles
NCH = N // 512      # 16 chunks of 512 tokens
TPC = 4             # token tiles per chunk
NT = N // 128       # 64 token tiles
LAG = 3             # chunks of slack between softmax chain and its PE consumers

_CACHE = {}


def _build_bass(reps=1, debug=False, phases=(1, 2, 3), hwloop=0, p3skip=(), p1mode='', body=1):
    import concourse.bacc as bacc
    import concourse.bass as bass
    import concourse.mybir as mybir
    import concourse.tile as tile

    f32 = mybir.dt.float32
    f32r = mybir.dt.float32r
    bf16 = mybir.dt.float16
    Alu = mybir.AluOpType
    Act = mybir.ActivationFunctionType

    nc = bacc.Bacc("TRN2", target_bir_lowering=False, debug=False, num_devices=B)

    xT = nc.dram_tensor("xT", [C, N], bf16, kind="ExternalInput")
    wqkvT = nc.dram_tensor("wqkvT", [C, C], bf16, kind="ExternalInput")
    woutT = nc.dram_tensor("woutT", [C, C], bf16, kind="ExternalInput")
    boutT = nc.dram_tensor("boutT", [128, CT], f32, kind="ExternalInput")
    maskf = nc.dram_tensor("maskf", [128, NT], f32, kind="ExternalInput")
    tempP = nc.dram_tensor("tempP", [128, CT], f32, kind="ExternalInput")
    identB = nc.dram_tensor("identB", [128, 128], bf16, kind="ExternalInput")
    ind8F = nc.dram_tensor("ind8F", [H, C], f32, kind="ExternalInput")
    outT = nc.dram_tensor("outT", [C, N], bf16, kind="ExternalOutput")
    if debug:
        dbg_inv = nc.dram_tensor("dbg_inv", [128, CT], f32, kind="ExternalOutput")
        dbg_spr = nc.dram_tensor("dbg_spr", [1, 16], f32, kind="ExternalOutput")
        dbg_watn = nc.dram_tensor("dbg_watn", [1, H], f32, kind="ExternalOutput")
        dbg_wex = nc.dram_tensor("dbg_wex", [128, CT], f32, kind="ExternalOutput")
        dbg_pi = nc.dram_tensor("dbg_pi", [128, NCH * TPC * H], bf16, kind="ExternalOutput")
        dbg_wt0 = nc.dram_tensor("dbg_wt0", [128, 512], bf16, kind="ExternalOutput")
        dbg_w20 = nc.dram_tensor("dbg_w20", [128, 512], bf16, kind="ExternalOutput")

    def r(ap):
        return ap.bitcast(f32r)

    with tile.TileContext(nc) as tc:
        with (
            tc.tile_pool(name="singles", bufs=1) as sing,
            tc.tile_pool(name="workB", bufs=3) as workB,
            tc.tile_pool(name="wts", bufs=12) as wtsp,
            tc.tile_pool(name="oc", bufs=3) as ocp,
            tc.tile_pool(name="small", bufs=2) as workS,
            tc.tile_pool(name="pirt", bufs=6) as pirtp,
            tc.tile_pool(name="pitc", bufs=3) as pitcp,
            tc.tile_pool(name="soft", bufs=6) as softp,
            tc.tile_pool(name="ps_big", bufs=5, space="PSUM") as ps_big,
            tc.tile_pool(name="ps_med", bufs=2, space="PSUM") as ps_med,
            tc.tile_pool(name="ps_s", bufs=1, space="PSUM") as ps_s,
        ):
            # ---------------- constants / persistent tiles ----------------
            wq = [sing.tile([128, C], bf16, tag=f"wq{i}", name=f"wq{i}") for i in range(CT)]
            wo = [sing.tile([128, C], bf16, tag=f"wo{i}", name=f"wo{i}") for i in range(CT)]
            wt = [sing.tile([128, N], bf16, tag=f"wt{i}", name=f"wt{i}") for i in range(CT)]
            w2 = [sing.tile([128, N], bf16, tag=f"w2_{i}", name=f"w2_{i}") for i in range(CT)]
            for i in range(CT):
                nc.sync.dma_start(out=wq[i][:], in_=wqkvT[i * 128:(i + 1) * 128, :])
            bout_sb = sing.tile([128, CT], f32, tag="bout", name="bout")
            maskf_sb = sing.tile([128, NT], f32, tag="maskf", name="maskf")
            tempP_sb = sing.tile([128, CT], f32, tag="tempP", name="tempP")
            identB_sb = sing.tile([128, 128], bf16, tag="identB", name="identB")
            ind8_sb = sing.tile([H, C], f32, tag="ind8F", name="ind8F")
            ind8H = sing.tile([H, C], bf16, tag="ind8H", name="ind8H")

            def preload_rest():
                # emitted mid-phase1 so the critical xt DMAs clear HWDGE first
                for i in range(CT):
                    nc.sync.dma_start(out=wo[i][:], in_=woutT[i * 128:(i + 1) * 128, :])
                nc.sync.dma_start(out=bout_sb[:], in_=boutT[:])
                nc.sync.dma_start(out=maskf_sb[:], in_=maskf[:])
                nc.sync.dma_start(out=tempP_sb[:], in_=tempP[:])
                nc.sync.dma_start(out=identB_sb[:], in_=identB[:])
                nc.sync.dma_start(out=ind8_sb[:], in_=ind8F[:])
                nc.vector.tensor_copy(ind8H[:], ind8_sb[:])

            ones1f = sing.tile([128, 1], f32, tag="ones1f", name="ones1f")
            nc.vector.memset(ones1f[:], 1.0)
            ones1b = sing.tile([128, 1], bf16, tag="ones1b", name="ones1b")
            nc.vector.tensor_copy(ones1b[:], ones1f[:])

            nsq = [sing.tile([128, NCH], f32, tag=f"nsq{i}", name=f"nsq{i}") for i in range(CT)]
            pi_all = sing.tile([128, NCH, TPC, H], bf16, tag="pi_all", name="pi_all")
            amat = [sing.tile([128, 16], bf16, tag=f"amat{i}", name=f"amat{i}") for i in range(CT)]
            inv = [sing.tile([128, 1], f32, tag=f"inv{i}", name=f"inv{i}") for i in range(CT)]
            wex = [sing.tile([128, 1], f32, tag=f"wex{i}", name=f"wex{i}") for i in range(CT)]
            spr = sing.tile([1, 16], f32, tag="spr", name="spr")
            watn = sing.tile([1, H], f32, tag="watn", name="watn")
            watnT = sing.tile([H, 1], f32, tag="watnT", name="watnT")

            psS_box = [None]
            psT2_box = [None]
            pitc2s = {}
            wtss = {}

            def phase1():
                for k in range(NCH):
                    if True:
                        xtb = workB.tile([128, CT, 512], bf16, tag="xtb", name="xtb")
                        if "nodma" not in p1mode:
                            xs = xT[0:128, k * 512:(k + 1) * 512]
                            bsrc = bass.AP(tensor=xs.tensor, offset=xs.offset,
                                           ap=[xs.ap[0], [128 * N, CT], xs.ap[1]])
                            nc.sync.dma_start(out=xtb[:], in_=bsrc)
                        xts = [xtb[:, ci, :] for ci in range(CT)]
                    # one PSUM bank at a time: evacuation of group co overlaps
                    # the matmuls of group co+1
                    for co in range(CT):
                        psA = ps_big.tile([128, 512], f32, tag="big", name="big")
                        for ci in range(CT):
                            nc.tensor.matmul(
                                psA[:], wq[ci][:, co * 128:(co + 1) * 128],
                                xts[ci],
                                start=(ci == 0), stop=(ci == CT - 1),
                            )
                        # DVE: evacuate w to resident bf16 wt
                        if "noevac" not in p1mode:
                            nc.vector.tensor_copy(
                                wt[co][:, k * 512:(k + 1) * 512], psA[:])
                        # ScalarE: square to bf16 w2; accum_out = per-channel
                        # sum of squares for this chunk (norm^2 partials)
                        if "nosq" not in p1mode:
                            nc.scalar.activation(
                                out=w2[co][:, k * 512:(k + 1) * 512], in_=psA[:],
                                func=Act.Square, accum_out=nsq[co][:, k:k + 1])
                        elif "noevac" in p1mode:
                            # keep one consumer so the bank frees deterministically
                            nc.scalar.activation(
                                out=w2[co][:, k * 512:(k + 1) * 512], in_=psA[0:128, 0:512],
                                func=Act.Copy)

            def norm_finalize():
                for ci in range(CT):
                    nsqt = workS.tile([128, 1], f32, tag="nsqt", name="nsqt")
                    nc.vector.reduce_sum(nsqt[:], nsq[ci][:], axis=mybir.AxisListType.X)
                    nc.vector.tensor_scalar_max(nsqt[:], nsqt[:], 1e-24)
                    nc.vector.reciprocal(inv[ci][:], nsqt[:])
                    am = workS.tile([128, 16], f32, tag="am_f", name="am_f")
                    nc.vector.memset(am[:], 0.0)
                    # col 2ci (rows 0:64) / col 2ci+1 (rows 64:128): inv * temp
                    nc.vector.tensor_copy(am[0:64, 2 * ci:2 * ci + 1], inv[ci][0:64, :])
                    nc.vector.tensor_copy(am[64:128, 2 * ci + 1:2 * ci + 2], inv[ci][64:128, :])
                    nc.vector.tensor_scalar_mul(
                        am[:, 0:H], am[:, 0:H], tempP_sb[:, ci:ci + 1])
                    nc.vector.memset(am[0:64, 8 + 2 * ci:8 + 2 * ci + 1], 1.0)
                    nc.vector.memset(am[64:128, 8 + 2 * ci + 1:8 + 2 * ci + 2], 1.0)
                    nc.vector.tensor_copy(amat[ci][:], am[:])

            def stageA2(j2):
                # Pi transposes for chunk pair j2 into one psT2 bank; pitc
                # evacuated per pair (head-major, bf16, unscaled)
                psT2 = ps_med.tile([H, 2 * 512], bf16, tag="med", name="psT2")
                for jh in range(2):
                    j = 2 * j2 + jh
                    off = jh * 512
                    for ti in range(TPC):
                        nc.tensor.transpose(
                            psT2[:, off + ti * 128:off + (ti + 1) * 128],
                            pi_all[:, j, ti, :], identB_sb[:])
                pitc2 = pitcp.tile([H, 2 * 512], bf16, tag="pitc", name="pitc")
                nc.scalar.activation(out=pitc2[:], in_=psT2[:], func=Act.Copy)
                pitc2s[j2] = pitc2

            def phase2a():
                psS_box[0] = ps_s.tile([1, 2 * TPC * H], f32, tag="psS", name="psS")
                psS = psS_box[0]
                # 8 accumulation streams share this bank; a start=True while
                # another stream is open wipes its partials (HW-verified), so
                # zero the bank once and accumulate with start=False only
                nc.vector.memset(psS[:], 0.0)
                pirts = {}
                erecs = {}

                def psS_mats(j):
                    # psS cols (ti,h): 0:32 <- sum Pi = erec^T ee,
                    # 32:64 <- sum Pi*r = erec^T (ee*r); erec is the stationary
                    # so nothing past the reciprocal gates the PE
                    ee, erec = erecs.pop(j)
                    pr = pirts.pop(j)
                    for ti in range(TPC):
                        nc.tensor.matmul(
                            psS[0:1, ti * H:(ti + 1) * H], erec[:, ti:ti + 1],
                            ee[:, ti, :], start=False, stop=(j == NCH - 1),
                            skip_group_check=True)
                        nc.tensor.matmul(
                            psS[0:1, TPC * H + ti * H:TPC * H + (ti + 1) * H],
                            erec[:, ti:ti + 1], pr[:, ti, :],
                            start=False, stop=(j == NCH - 1),
                            skip_group_check=True)

                for k in range(NCH):
                    psB = ps_med.tile([128, TPC, 16], f32, tag="med", name="psB")
                    for ti in range(TPC):
                        t = k * TPC + ti
                        for ci in range(CT):
                            nc.tensor.matmul(
                                psB[:, ti, :],
                                w2[ci][:, t * 128:(t + 1) * 128],
                                amat[ci][:],
                                start=(ci == 0), stop=(ci == CT - 1))
                    # copy r out first (ScalarE, no deps) so psB frees fast
                    rc = softp.tile([128, TPC, H], f32, tag="rc", name="rc")
                    nc.scalar.activation(out=rc[:], in_=psB[:, :, 8:16], func=Act.Copy)
                    # logits = sum_ws*temp*mask: with temp==ones a masked token
                    # gets all-zero logits -> exactly the reference's uniform
                    # 1/8 softmax, so no mask bias is needed at all.
                    lg = softp.tile([128, TPC, H], f32, tag="lg", name="lg")
                    mf = maskf_sb[:, k * TPC:(k + 1) * TPC]
                    mfb = bass.AP(tensor=mf.tensor, offset=mf.offset,
                                  ap=[mf.ap[0], mf.ap[1], [0, H]])
                    nc.vector.tensor_mul(lg[:], psB[:, :, 0:H], mfb)
                    nc.scalar.activation(out=lg[:], in_=lg[:], func=Act.Exp)
                    # pirt_raw = ee*r runs off the chain critical path
                    pirt = pirtp.tile([128, TPC, H], f32, tag="pirt", name="pirt")
                    pirts[k] = pirt
                    nc.vector.tensor_mul(pirt[:], lg[:], rc[:])
                    erec = softp.tile([128, TPC], f32, tag="erec", name="erec")
                    nc.vector.reduce_sum(erec[:], lg[:], axis=mybir.AxisListType.X)
                    nc.vector.reciprocal(erec[:], erec[:])
                    erecs[k] = (lg, erec)
                    er = erec[:]
                    erb = bass.AP(tensor=er.tensor, offset=er.offset,
                                  ap=[er.ap[0], er.ap[1], [0, H]])
                    nc.vector.tensor_mul(pi_all[:, k, :, :], lg[:], erb)
                    # PE consumers of the softmax chain lag LAG chunks so the
                    # tensor engine never stalls on it
                    if k >= LAG:
                        psS_mats(k - LAG)
                for j in range(NCH - LAG, NCH):
                    psS_mats(j)

            def global_scalars():
                psS = psS_box[0]
                # spr[0,0:8] = S[h], spr[0,8:16] = PR[h]
                nc.vector.reduce_sum(
                    spr[:].rearrange("p (g h) -> p g h", g=2),
                    psS[:].rearrange("p (g t h) -> p g h t", g=2, t=TPC, h=H),
                    axis=mybir.AxisListType.X)
                srec = workS.tile([1, H], f32, tag="srec", name="srec")
                nc.vector.tensor_scalar_add(srec[:], spr[0:1, 0:H], 1e-8)
                nc.vector.reciprocal(srec[:], srec[:])
                dots = workS.tile([1, H], f32, tag="dots", name="dots")
                nc.vector.tensor_mul(dots[:], spr[0:1, H:2 * H], srec[:])
                nc.vector.tensor_scalar_add(dots[:], dots[:], 1.0)
                nc.vector.reciprocal(watn[:], dots[:])
                nc.vector.tensor_scalar_mul(watn[:], watn[:], -1.0)
                psW = ps_med.tile([H, 1], f32, tag="med", name="psW")
                nc.tensor.matmul(psW[:], watn[:], ones1f[0:1, 0:1], is_transpose=True)
                nc.scalar.activation(out=watnT[:], in_=psW[:], func=Act.Copy)
                # wex[ci][p] = -attn[h(p, ci)]: expand across channel partitions
                for ci in range(CT):
                    psWE = ps_med.tile([128, 1], f32, tag="med", name="psWE")
                    nc.tensor.matmul(
                        psWE[:], ind8_sb[:, ci * 128:(ci + 1) * 128], watnT[:],
                        start=True, stop=True)
                    nc.scalar.activation(out=wex[ci][:], in_=psWE[:], func=Act.Copy)

            def stageB(k):
                # psE[c, n] = Pi[h(c), n] by a PE indicator matmul (the
                # broadcast DMA alternative measures ~8us per transfer), then
                # wts = wt * (-attn[h(c)]) * Pi in one DVE op
                pitc2 = pitc2s[k // 2]
                jh = k % 2
                if jh == 1:
                    del pitc2s[k // 2]
                wts = []
                for ci in range(CT):
                    psE = ps_big.tile([128, 512], f32, tag="big", name="bigE")
                    nc.tensor.matmul(
                        psE[:], ind8H[:, ci * 128:(ci + 1) * 128],
                        pitc2[:, jh * 512:(jh + 1) * 512],
                        start=True, stop=True)
                    w = wtsp.tile([128, 512], bf16, tag="wts", name="wts")
                    nc.vector.scalar_tensor_tensor(
                        out=w[:], in0=wt[ci][:, k * 512:(k + 1) * 512],
                        scalar=wex[ci][:], in1=psE[:],
                        op0=Alu.mult, op1=Alu.mult)
                    wts.append(w)
                wtss[k] = wts

            def stageC(k):
                wts = wtss.pop(k)
                oc = ocp.tile([128, CT, 512], bf16, tag="outc", name="outc")
                for oj in range(CT):
                    psC = ps_big.tile([128, 512], f32, tag="big", name="bigC")
                    for ci in range(CT):
                        nc.tensor.matmul(
                            psC[:], wo[ci][:, oj * 128:(oj + 1) * 128],
                            wts[ci][:],
                            start=(ci == 0), stop=(ci == CT - 1))
                    nc.scalar.activation(
                        out=oc[:, oj, :], in_=psC[:], func=Act.Identity,
                        bias=bout_sb[:, oj:oj + 1], scale=1.0)
                # one batched output DMA per chunk
                if "odma" not in p3skip:
                    od = outT[0:128, k * 512:(k + 1) * 512]
                    dst = bass.AP(tensor=od.tensor, offset=od.offset,
                                  ap=[od.ap[0], [128 * N, CT], od.ap[1]])
                    # Activation HWDGE queue: keeps output stores from
                    # head-of-line blocking the next rep's xt prefetch on SP
                    nc.scalar.dma_start(out=dst, in_=oc[:])

            def phase3():
                stageB(0)
                stageB(1)
                for k in range(NCH):
                    if k % 2 == 0 and k // 2 + 2 < NCH // 2:
                        stageA2(k // 2 + 2)
                    if k + 2 < NCH:
                        stageB(k + 2)
                    stageC(k)

            def rep_body(first):
                if 1 in phases:
                    phase1()
                if first:
                    preload_rest()
                if 1 in phases:
                    norm_finalize()
                if 2 in phases:
                    phase2a()
                    # PE runs these transposes while DVE computes the attn
                    # scalars chain (in-order PE queue would otherwise stall
                    # at the psW matmul)
                    stageA2(0)
                    stageA2(1)
                    global_scalars()
                if 3 in phases:
                    phase3()

            if hwloop > 1:
                rep_body(True)
                with tc.For_i(0, (hwloop - 1) // body, 1):
                    for _ in range(body):
                        rep_body(False)
            else:
                for _rep in range(reps):
                    rep_body(_rep == 0)
            if debug:
                for ci in range(CT):
                    nc.sync.dma_start(out=dbg_inv[:, ci:ci + 1], in_=inv[ci][:])
                    nc.sync.dma_start(out=dbg_wex[:, ci:ci + 1], in_=wex[ci][:])
                nc.sync.dma_start(out=dbg_spr[:], in_=spr[:])
                nc.sync.dma_start(out=dbg_watn[:], in_=watn[:])
                nc.sync.dma_start(out=dbg_pi[:], in_=pi_all[:].rearrange("p a b c -> p (a b c)"))
                nc.sync.dma_start(out=dbg_wt0[:], in_=wt[0][:, 0:512])
                nc.sync.dma_start(out=dbg_w20[:], in_=w2[0][:, 0:512])

    nc.compile()
    return nc


def _prep_inputs(x, token_mask, Wqkv, temp, Wout, bout):
    f = np.float32
    bf = np.float16
    temp = np.asarray(temp, dtype=f)
    wqkvT = np.ascontiguousarray(np.asarray(Wqkv, f).T.astype(bf))
    woutT = np.ascontiguousarray(np.asarray(Wout, f).T.astype(bf))
    boutT = np.ascontiguousarray(np.asarray(bout, f).reshape(CT, 128).T)
    identB = np.eye(128, dtype=bf)
    ind8F = (np.arange(C) // D == np.arange(H)[:, None]).astype(f)
    # tempP[p, ci] = temp[2ci + (p>=64)]
    tempP = np.empty((128, CT), f)
    for ci in range(CT):
        tempP[0:64, ci] = temp[2 * ci, 0]
        tempP[64:128, ci] = temp[2 * ci + 1, 0]
    in_maps = []
    for b in range(B):
        m = np.asarray(token_mask[b], f)          # [N]
        mt = m.reshape(NT, 128).T.copy()          # [128, NT]
        in_maps.append({
            "xT": np.ascontiguousarray(np.asarray(x[b], f).T.astype(bf)),
            "wqkvT": wqkvT,
            "woutT": woutT,
            "boutT": boutT,
            "maskf": mt,
            "tempP": tempP,
            "identB": identB,
            "ind8F": ind8F,
        })
    return in_maps


def kernel(**inputs):
    from concourse.bass_utils import run_bass_kernel_spmd

    if "nc" not in _CACHE:
        _CACHE["nc"] = _build_bass()
    nc = _CACHE["nc"]
    in_maps = _prep_inputs(**inputs)
    try:
        res = run_bass_kernel_spmd(nc, in_maps, core_ids=list(range(B)))
    except Exception:
        # transient device/tunnel hiccup: retry once
        import time as _t
        _t.sleep(2.0)
        res = run_bass_kernel_spmd(nc, in_maps, core_ids=list(range(B)))
    out = np.empty((B, N, C), np.float32)
    for b in range(B):
        out[b] = np.asarray(res.results[b]["outT"], dtype=np.float32).T
    return out


# revision 45
# speedup vs baseline: 2.9704x; 2.9704x over previous
"""Trainium2 Bass kernel for nn_AttentionTSSA (B=8, N=8192, C=512, H=8).

Sharding: data-parallel over batch B across the 8 NeuronCores (1 batch each,
no collectives).  Per core (all 16-bit tensors are fp16: same engine speeds
as bf16 but 8x the mantissa — total rel err ~7e-4):

  phase 1:  wT[c, n] = Wqkv @ x^T (fp16 GEMM, 512-token chunks, one batched
            x DMA per chunk).  Each PSUM bank is evacuated twice: DVE copies
            to fp16 wt (kept resident), ScalarE squares to fp16 w2 with
            accum_out giving per-channel norm^2 partials for free.
  finalize: inv[c] = 1/max(norm^2, eps); amat[ci] [128,16] fp16 packs
            inv*temp (cols 0:8) and ones (cols 8:16).
  phase 2a: per 128-token tile, matmul(w2 tile, amat) -> psB[tok, 16] =
            (sum_ws*temp | r).  With temp==ones the mask multiply alone
            reproduces the reference masked softmax (uniform 1/8), so the
            softmax is: one masked TT, one Exp, reduce, recip, one TT.
            sum(Pi) / sum(Pi*r) accumulate into a single psS bank via
            erec-stationary matmuls (the bank is memset once and accumulated
            with start=False only: an interleaved start=True wipes other
            open groups' partials in the same bank — HW-verified).  The PE
            consumers of the softmax chain lag LAG chunks so PE never stalls.
  scalars:  dots/attn from psS; -attn[h] expanded per channel partition via
            tiny ind8 matmuls (wex[ci]).
  phase 3:  per 2-chunk pair, Pi transposes to head-major pitc; per chunk,
            psE[c, n] = Pi[h(c), n] by a PE indicator matmul (a partition-
            broadcast DMA measures ~8us/transfer on this HW — avoid), then
            wts = wt * (-attn[h(c)]) * Pi in one scalar_tensor_tensor; the
            output GEMM outT = WoutT^T @ wts with bias fused into the PSUM
            evacuation; one batched fp16 output DMA per chunk (host upcasts).

Timing note: unrolled multi-rep NEFFs are instruction-fetch bound on this
part (~4x slowdown at 33 reps); test.py measures with an on-device tc.For_i
hardware loop instead.

Host side transposes x per batch and un-transposes/upcasts the outputs.
"""

import numpy as np

B, N, C, H = 8, 8192, 512, 8
D = C // H          # 64
CT = C // 128       # 4 channel tiles
NCH = N // 512      # 16 chunks of 512 tokens
TPC = 4             # token tiles per chunk
NT = N // 128       # 64 token tiles
LAG = 3             # chunks of slack between softmax chain and its PE consumers

_CACHE = {}


def _build_bass(reps=1, debug=False, phases=(1, 2, 3), hwloop=0, p3skip=(), p1mode='', body=1):
    import concourse.bacc as bacc
    import concourse.bass as bass
    import concourse.mybir as mybir
    import concourse.tile as tile

    f32 = mybir.dt.float32
    f32r = mybir.dt.float32r
    bf16 = mybir.dt.float16
    Alu = mybir.AluOpType
    Act = mybir.ActivationFunctionType

    nc = bacc.Bacc("TRN2", target_bir_lowering=False, debug=False, num_devices=B)

    xT = nc.dram_tensor("xT", [C, N], bf16, kind="ExternalInput")
    wqkvT = nc.dram_tensor("wqkvT", [C, C], bf16, kind="ExternalInput")
    woutT = nc.dram_tensor("woutT", [C, C], bf16, kind="ExternalInput")
    boutT = nc.dram_tensor("boutT", [128, CT], f32, kind="ExternalInput")
    maskf = nc.dram_tensor("maskf", [128, NT], f32, kind="ExternalInput")
    tempP = nc.dram_tensor("tempP", [128, CT], f32, kind="ExternalInput")
    identB = nc.dram_tensor("identB", [128, 128], bf16, kind="ExternalInput")
    ind8F = nc.dram_tensor("ind8F", [H, C], f32, kind="ExternalInput")
    outT = nc.dram_tensor("outT", [C, N], bf16, kind="ExternalOutput")
    if debug:
        dbg_inv = nc.dram_tensor("dbg_inv", [128, CT], f32, kind="ExternalOutput")
        dbg_spr = nc.dram_tensor("dbg_spr", [1, 16], f32, kind="ExternalOutput")
        dbg_watn = nc.dram_tensor("dbg_watn", [1, H], f32, kind="ExternalOutput")
        dbg_wex = nc.dram_tensor("dbg_wex", [128, CT], f32, kind="ExternalOutput")
        dbg_pi = nc.dram_tensor("dbg_pi", [128, NCH * TPC * H], bf16, kind="ExternalOutput")
        dbg_wt0 = nc.dram_tensor("dbg_wt0", [128, 512], bf16, kind="ExternalOutput")
        dbg_w20 = nc.dram_tensor("dbg_w20", [128, 512], bf16, kind="ExternalOutput")

    def r(ap):
        return ap.bitcast(f32r)

    with tile.TileContext(nc) as tc:
        with (
            tc.tile_pool(name="singles", bufs=1) as sing,
            tc.tile_pool(name="workB", bufs=3) as workB,
            tc.tile_pool(name="wts", bufs=12) as wtsp,
            tc.tile_pool(name="oc", bufs=3) as ocp,
            tc.tile_pool(name="small", bufs=2) as workS,
            tc.tile_pool(name="pirt", bufs=6) as pirtp,
            tc.tile_pool(name="pitc", bufs=3) as pitcp,
            tc.tile_pool(name="soft", bufs=6) as softp,
            tc.tile_pool(name="ps_big", bufs=5, space="PSUM") as ps_big,
            tc.tile_pool(name="ps_med", bufs=2, space="PSUM") as ps_med,
            tc.tile_pool(name="ps_s", bufs=1, space="PSUM") as ps_s,
        ):
            # ---------------- constants / persistent tiles ----------------
            wq = [sing.tile([128, C], bf16, tag=f"wq{i}", name=f"wq{i}") for i in range(CT)]
            wo = [sing.tile([128, C], bf16, tag=f"wo{i}", name=f"wo{i}") for i in range(CT)]
            wt = [sing.tile([128, N], bf16, tag=f"wt{i}", name=f"wt{i}") for i in range(CT)]
            w2 = [sing.tile([128, N], bf16, tag=f"w2_{i}", name=f"w2_{i}") for i in range(CT)]
            for i in range(CT):
                nc.sync.dma_start(out=wq[i][:], in_=wqkvT[i * 128:(i + 1) * 128, :])
            bout_sb = sing.tile([128, CT], f32, tag="bout", name="bout")
            maskf_sb = sing.tile([128, NT], f32, tag="maskf", name="maskf")
            tempP_sb = sing.tile([128, CT], f32, tag="tempP", name="tempP")
            identB_sb = sing.tile([128, 128], bf16, tag="identB", name="identB")
            ind8_sb = sing.tile([H, C], f32, tag="ind8F", name="ind8F")
            ind8H = sing.tile([H, C], bf16, tag="ind8H", name="ind8H")

            def preload_rest():
                # emitted mid-phase1 so the critical xt DMAs clear HWDGE first
                for i in range(CT):
                    nc.sync.dma_start(out=wo[i][:], in_=woutT[i * 128:(i + 1) * 128, :])
                nc.sync.dma_start(out=bout_sb[:], in_=boutT[:])
                nc.sync.dma_start(out=maskf_sb[:], in_=maskf[:])
                nc.sync.dma_start(out=tempP_sb[:], in_=tempP[:])
                nc.sync.dma_start(out=identB_sb[:], in_=identB[:])
                nc.sync.dma_start(out=ind8_sb[:], in_=ind8F[:])
                nc.vector.tensor_copy(ind8H[:], ind8_sb[:])

            ones1f = sing.tile([128, 1], f32, tag="ones1f", name="ones1f")
            nc.vector.memset(ones1f[:], 1.0)
            ones1b = sing.tile([128, 1], bf16, tag="ones1b", name="ones1b")
            nc.vector.tensor_copy(ones1b[:], ones1f[:])

            nsq = [sing.tile([128, NCH], f32, tag=f"nsq{i}", name=f"nsq{i}") for i in range(CT)]
            pi_all = sing.tile([128, NCH, TPC, H], bf16, tag="pi_all", name="pi_all")
            amat = [sing.tile([128, 16], bf16, tag=f"amat{i}", name=f"amat{i}") for i in range(CT)]
            inv = [sing.tile([128, 1], f32, tag=f"inv{i}", name=f"inv{i}") for i in range(CT)]
            wex = [sing.tile([128, 1], f32, tag=f"wex{i}", name=f"wex{i}") for i in range(CT)]
            spr = sing.tile([1, 16], f32, tag="spr", name="spr")
            watn = sing.tile([1, H], f32, tag="watn", name="watn")
            watnT = sing.tile([H, 1], f32, tag="watnT", name="watnT")

            psS_box = [None]
            psT2_box = [None]
            pitc2s = {}
            wtss = {}

            def phase1():
                for k in range(NCH):
                    if True:
                        xtb = workB.tile([128, CT, 512], bf16, tag="xtb", name="xtb")
                        if "nodma" not in p1mode:
                            xs = xT[0:128, k * 512:(k + 1) * 512]
                            bsrc = bass.AP(tensor=xs.tensor, offset=xs.offset,
                                           ap=[xs.ap[0], [128 * N, CT], xs.ap[1]])
                            nc.sync.dma_start(out=xtb[:], in_=bsrc)
                        xts = [xtb[:, ci, :] for ci in range(CT)]
                    # one PSUM bank at a time: evacuation of group co overlaps
                    # the matmuls of group co+1
                    for co in range(CT):
                        psA = ps_big.tile([128, 512], f32, tag="big", name="big")
                        for ci in range(CT):
                            nc.tensor.matmul(
                                psA[:], wq[ci][:, co * 128:(co + 1) * 128],
                                xts[ci],
                                start=(ci == 0), stop=(ci == CT - 1),
                            )
                        # DVE: evacuate w to resident bf16 wt
                        if "noevac" not in p1mode:
                            nc.vector.tensor_copy(
                                wt[co][:, k * 512:(k + 1) * 512], psA[:])
                        # ScalarE: square to bf16 w2; accum_out = per-channel
                        # sum of squares for this chunk (norm^2 partials)
                        if "nosq" not in p1mode:
                            nc.scalar.activation(
                                out=w2[co][:, k * 512:(k + 1) * 512], in_=psA[:],
                                func=Act.Square, accum_out=nsq[co][:, k:k + 1])
                        elif "noevac" in p1mode:
                            # keep one consumer so the bank frees deterministically
                            nc.scalar.activation(
                                out=w2[co][:, k * 512:(k + 1) * 512], in_=psA[0:128, 0:512],
                                func=Act.Copy)

            def norm_finalize():
                for ci in range(CT):
                    nsqt = workS.tile([128, 1], f32, tag="nsqt", name="nsqt")
                    nc.vector.reduce_sum(nsqt[:], nsq[ci][:], axis=mybir.AxisListType.X)
                    nc.vector.tensor_scalar_max(nsqt[:], nsqt[:], 1e-24)
                    nc.vector.reciprocal(inv[ci][:], nsqt[:])
                    am = workS.tile([128, 16], f32, tag="am_f", name="am_f")
                    nc.vector.memset(am[:], 0.0)
                    # col 2ci (rows 0:64) / col 2ci+1 (rows 64:128): inv * temp
                    nc.vector.tensor_copy(am[0:64, 2 * ci:2 * ci + 1], inv[ci][0:64, :])
                    nc.vector.tensor_copy(am[64:128, 2 * ci + 1:2 * ci + 2], inv[ci][64:128, :])
                    nc.vector.tensor_scalar_mul(
                        am[:, 0:H], am[:, 0:H], tempP_sb[:, ci:ci + 1])
                    nc.vector.memset(am[0:64, 8 + 2 * ci:8 + 2 * ci + 1], 1.0)
                    nc.vector.memset(am[64:128, 8 + 2 * ci + 1:8 + 2 * ci + 2], 1.0)
                    nc.vector.tensor_copy(amat[ci][:], am[:])

            def stageA2(j2):
                # Pi transposes for chunk pair j2 into one psT2 bank; pitc
                # evacuated per pair (head-major, bf16, unscaled)
                psT2 = ps_med.tile([H, 2 * 512], bf16, tag="med", name="psT2")
                for jh in range(2):
                    j = 2 * j2 + jh
                    off = jh * 512
                    for ti in range(TPC):
                        nc.tensor.transpose(
                            psT2[:, off + ti * 128:off + (ti + 1) * 128],
                            pi_all[:, j, ti, :], identB_sb[:])
                pitc2 = pitcp.tile([H, 2 * 512], bf16, tag="pitc", name="pitc")
                nc.scalar.activation(out=pitc2[:], in_=psT2[:], func=Act.Copy)
                pitc2s[j2] = pitc2

            def phase2a():
                psS_box[0] = ps_s.tile([1, 2 * TPC * H], f32, tag="psS", name="psS")
                psS = psS_box[0]
                # 8 accumulation streams share this bank; a start=True while
                # another stream is open wipes its partials (HW-verified), so
                # zero the bank once and accumulate with start=False only
                nc.vector.memset(psS[:], 0.0)
                pirts = {}
                erecs = {}

                def psS_mats(j):
                    # psS cols (ti,h): 0:32 <- sum Pi = erec^T ee,
                    # 32:64 <- sum Pi*r = erec^T (ee*r); erec is the stationary
                    # so nothing past the reciprocal gates the PE
                    ee, erec = erecs.pop(j)
                    pr = pirts.pop(j)
                    for ti in range(TPC):
                        nc.tensor.matmul(
                            psS[0:1, ti * H:(ti + 1) * H], erec[:, ti:ti + 1],
                            ee[:, ti, :], start=False, stop=(j == NCH - 1),
                            skip_group_check=True)
                        nc.tensor.matmul(
                            psS[0:1, TPC * H + ti * H:TPC * H + (ti + 1) * H],
                            erec[:, ti:ti + 1], pr[:, ti, :],
                            start=False, stop=(j == NCH - 1),
                            skip_group_check=True)

                for k in range(NCH):
                    psB = ps_med.tile([128, TPC, 16], f32, tag="med", name="psB")
                    for ti in range(TPC):
                        t = k * TPC + ti
                        for ci in range(CT):
                            nc.tensor.matmul(
                                psB[:, ti, :],
                                w2[ci][:, t * 128:(t + 1) * 128],
                                amat[ci][:],
                                start=(ci == 0), stop=(ci == CT - 1))
                    # copy r out first (ScalarE, no deps) so psB frees fast
                    rc = softp.tile([128, TPC, H], f32, tag="rc", name="rc")
                    nc.scalar.activation(out=rc[:], in_=psB[:, :, 8:16], func=Act.Copy)
                    # logits = sum_ws*temp*mask: with temp==ones a masked token
                    # gets all-zero logits -> exactly the reference's uniform
                    # 1/8 softmax, so no mask bias is needed at all.
                    lg = softp.tile([128, TPC, H], f32, tag="lg", name="lg")
                    mf = maskf_sb[:, k * TPC:(k + 1) * TPC]
                    mfb = bass.AP(tensor=mf.tensor, offset=mf.offset,
                                  ap=[mf.ap[0], mf.ap[1], [0, H]])
                    nc.vector.tensor_mul(lg[:], psB[:, :, 0:H], mfb)
                    nc.scalar.activation(out=lg[:], in_=lg[:], func=Act.Exp)
                    # pirt_raw = ee*r runs off the chain critical path
                    pirt = pirtp.tile([128, TPC, H], f32, tag="pirt", name="pirt")
                    pirts[k] = pirt
                    nc.vector.tensor_mul(pirt[:], lg[:], rc[:])
                    erec = softp.tile([128, TPC], f32, tag="erec", name="erec")
                    nc.vector.reduce_sum(erec[:], lg[:], axis=mybir.AxisListType.X)
                    nc.vector.reciprocal(erec[:], erec[:])
                    erecs[k] = (lg, erec)
                    er = erec[:]
                    erb = bass.AP(tensor=er.tensor, offset=er.offset,
                                  ap=[er.ap[0], er.ap[1], [0, H]])
                    nc.vector.tensor_mul(pi_all[:, k, :, :], lg[:], erb)
                    # PE consumers of the softmax chain lag LAG chunks so the
                    # tensor engine never stalls on it
                    if k >= LAG:
                        psS_mats(k - LAG)
                for j in range(NCH - LAG, NCH):
                    psS_mats(j)

            def global_scalars():
                psS = psS_box[0]
                # spr[0,0:8] = S[h], spr[0,8:16] = PR[h]
                nc.vector.reduce_sum(
                    spr[:].rearrange("p (g h) -> p g h", g=2),
                    psS[:].rearrange("p (g t h) -> p g h t", g=2, t=TPC, h=H),
                    axis=mybir.AxisListType.X)
                srec = workS.tile([1, H], f32, tag="srec", name="srec")
                nc.vector.tensor_scalar_add(srec[:], spr[0:1, 0:H], 1e-8)
                nc.vector.reciprocal(srec[:], srec[:])
                dots = workS.tile([1, H], f32, tag="dots", name="dots")
                nc.vector.tensor_mul(dots[:], spr[0:1, H:2 * H], srec[:])
                nc.vector.tensor_scalar_add(dots[:], dots[:], 1.0)
                nc.vector.reciprocal(watn[:], dots[:])
                nc.vector.tensor_scalar_mul(watn[:], watn[:], -1.0)
                psW = ps_med.tile([H, 1], f32, tag="med", name="psW")
                nc.tensor.matmul(psW[:], watn[:], ones1f[0:1, 0:1], is_transpose=True)
                nc.scalar.activation(out=watnT[:], in_=psW[:], func=Act.Copy)
                # wex[ci][p] = -attn[h(p, ci)]: expand across channel partitions
                for ci in range(CT):
                    psWE = ps_med.tile([128, 1], f32, tag="med", name="psWE")
                    nc.tensor.matmul(
                        psWE[:], ind8_sb[:, ci * 128:(ci + 1) * 128], watnT[:],
                        start=True, stop=True)
                    nc.scalar.activation(out=wex[ci][:], in_=psWE[:], func=Act.Copy)

            def stageB(k):
                # psE[c, n] = Pi[h(c), n] by a PE indicator matmul (the
                # broadcast DMA alternative measures ~8us per transfer), then
                # wts = wt * (-attn[h(c)]) * Pi in one DVE op
                pitc2 = pitc2s[k // 2]
                jh = k % 2
                if jh == 1:
                    del pitc2s[k // 2]
                wts = []
                for ci in range(CT):
                    psE = ps_big.tile([128, 512], f32, tag="big", name="bigE")
                    nc.tensor.matmul(
                        psE[:], ind8H[:, ci * 128:(ci + 1) * 128],
                        pitc2[:, jh * 512:(jh + 1) * 512],
                        start=True, stop=True)
                    w = wtsp.tile([128, 512], bf16, tag="wts", name="wts")
                    nc.vector.scalar_tensor_tensor(
                        out=w[:], in0=wt[ci][:, k * 512:(k + 1) * 512],
                        scalar=wex[ci][:], in1=psE[:],
                        op0=Alu.mult, op1=Alu.mult)
                    wts.append(w)
                wtss[k] = wts

            def stageC(k):
                wts = wtss.pop(k)
                oc = ocp.tile([128, CT, 512], bf16, tag="outc", name="outc")
                for oj in range(CT):
                    psC = ps_big.tile([128, 512], f32, tag="big", name="bigC")
                    for ci in range(CT):
                        nc.tensor.matmul(
                            psC[:], wo[ci][:, oj * 128:(oj + 1) * 128],
                            wts[ci][:],
                            start=(ci == 0), stop=(ci == CT - 1))
                    nc.scalar.activation(
                        out=oc[:, oj, :], in_=psC[:], func=Act.Identity,
                        bias=bout_sb[:, oj:oj + 1], scale=1.0)
                # one batched output DMA per chunk
                if "odma" not in p3skip:
                    od = outT[0:128, k * 512:(k + 1) * 512]
                    dst = bass.AP(tensor=od.tensor, offset=od.offset,
                                  ap=[od.ap[0], [128 * N, CT], od.ap[1]])
                    nc.sync.dma_start(out=dst, in_=oc[:])

            def phase3():
                stageB(0)
                stageB(1)
                for k in range(NCH):
                    if k % 2 == 0 and k // 2 + 2 < NCH // 2:
                        stageA2(k // 2 + 2)
                    if k + 2 < NCH:
                        stageB(k + 2)
                    stageC(k)

            def rep_body(first):
                if 1 in phases:
                    phase1()
                if first:
                    preload_rest()
                if 1 in phases:
                    norm_finalize()
                if 2 in phases:
                    phase2a()
                    # PE runs these transposes while DVE computes the attn
                    # scalars chain (in-order PE queue would otherwise stall
                    # at the psW matmul)
                    stageA2(0)
                    stageA2(1)
                    global_scalars()
                if 3 in phases:
                    phase3()

            if hwloop > 1:
                rep_body(True)
                with tc.For_i(0, (hwloop - 1) // body, 1):
                    for _ in range(body):
                        rep_body(False)
            else:
                for _rep in range(reps):
                    rep_body(_rep == 0)
            if debug:
                for ci in range(CT):
                    nc.sync.dma_start(out=dbg_inv[:, ci:ci + 1], in_=inv[ci][:])
                    nc.sync.dma_start(out=dbg_wex[:, ci:ci + 1], in_=wex[ci][:])
                nc.sync.dma_start(out=dbg_spr[:], in_=spr[:])
                nc.sync.dma_start(out=dbg_watn[:], in_=watn[:])
                nc.sync.dma_start(out=dbg_pi[:], in_=pi_all[:].rearrange("p a b c -> p (a b c)"))
                nc.sync.dma_start(out=dbg_wt0[:], in_=wt[0][:, 0:512])
                nc.sync.dma_start(out=dbg_w20[:], in_=w2[0][:, 0:512])

    nc.compile()
    return nc


def _prep_inputs(x, token_mask, Wqkv, temp, Wout, bout):
    f = np.float32
    bf = np.float16
    temp = np.asarray(temp, dtype=f)
    wqkvT = np.ascontiguousarray(np.asarray(Wqkv, f).T.astype(bf))
    woutT = np.ascontiguousarray(np.asarray(Wout, f).T.astype(bf))
    boutT = np.ascontiguousarray(np.asarray(bout, f).reshape(CT, 128).T)
    identB = np.eye(128, dtype=bf)
    ind8F = (np.arange(C) // D == np.arange(H)[:, None]).astype(f)
    # tempP[p, ci] = temp[2ci + (p>=64)]
    tempP = np.empty((128, CT), f)
    for ci in range(CT):
        tempP[0:64, ci] = temp[2 * ci, 0]
        tempP[64:128, ci] = temp[2 * ci + 1, 0]
    in_maps = []
    for b in range(B):
        m = np.asarray(token_mask[b], f)          # [N]
        mt = m.reshape(NT, 128).T.copy()          # [128, NT]
        in_maps.append({
            "xT": np.ascontiguousarray(np.asarray(x[b], f).T.astype(bf)),
            "wqkvT": wqkvT,
            "woutT": woutT,
            "boutT": boutT,
            "maskf": mt,
            "tempP": tempP,
            "identB": identB,
            "ind8F": ind8F,
        })
    return in_maps


def kernel(**inputs):
    from concourse.bass_utils import run_bass_kernel_spmd

    if "nc" not in _CACHE:
        _CACHE["nc"] = _build_bass()
    nc = _CACHE["nc"]
    in_maps = _prep_inputs(**inputs)
    try:
        res = run_bass_kernel_spmd(nc, in_maps, core_ids=list(range(B)))
    except Exception:
        # transient device/tunnel hiccup: retry once
        import time as _t
        _t.sleep(2.0)
        res = run_bass_kernel_spmd(nc, in_maps, core_ids=list(range(B)))
    out = np.empty((B, N, C), np.float32)
    for b in range(B):
        out[b] = np.asarray(res.results[b]["outT"], dtype=np.float32).T
    return out
